# revision 8
# baseline (speedup 1.0000x reference)
"""Trainium2 Bass kernel for nn_DiagonalTraining (ragged per-anti-diagonal linear).

Math (reference): for each batch image x[b] (SxS) and each anti-diagonal
i (elements x[b, r, i-r], r=0..i), apply a per-diagonal linear layer:
  out[b,i,q] = sum_{r<=i} x[b,r,i-r] * W[i,q,r] + bias[i,q]   (q <= i)
and scatter back: y[b,q,i-q] = out[b,i,q]; positions with r+c >= S keep x.

Distribution: diagonal i -> core i%8, slot j=i//8 (64 slots per core,
balanced by construction). Host packs, per (core, slot), an augmented
matrix whose rows are the contraction axis r:
  [ D^T | V ]  with D^T[r,b]=x[b,r,i-r], V[r,q]=W[i,q,r]  (r,q < ni=i+1)
zero-padded to a core-independent size NJ=8*(j+1) (>= ni for every
core) so the SPMD program is identical on all cores. The per-diagonal
bias is added on the host while scattering results back (elementwise,
~0.05% of the FLOPs; the whole einsum runs on device).

Device ("window streaming"): each slot is split into row-chunks of up
to 128 rows; chunk columns ([*, 32+NJ] blocks) are packed first-fit
into uniform [128, WF] window tiles. Partial (<128-row) chunks from
different slots share one column block vertically at matmul-legal PE
tile offsets (rows<=32 at partitions {0,32,64,96}, <=64 at {0,64}),
eliminating most row-padding HBM traffic. The windows are loaded by
identical big SWDGE DMAs (128 descriptors of WF*dtype bytes each) —
full 128-partition DMAs spread evenly over all 16 SDMA engines that
stream at near-HBM rate, fully decoupled from compute. Matmuls read
chunks at static (window, column, partition) offsets, accumulating
psum[32, NJ] per slot inside a bank-packed 4-slot group psum tile; one
DVE copy per group stages results, and all group stores run at the end
of the SWDGE queue.

Only the live (lower-triangular) part of W is shipped/read, in bf16
(~13 MB/core vs 512 MB full f32 W) — the kernel is HBM-bound on ~those
bytes. bf16 matmul streams 1 column/cycle; rel-err stays ~1e-3 vs the
2e-2 gate (products accumulate in f32 PSUM).
"""

import sys

for _p in ("/opt/trn_rl_repo", "/opt/pypackages"):
    if _p not in sys.path:
        sys.path.append(_p)

import numpy as np

import concourse.bass as bass  # noqa: F401
import concourse.tile as tile
from concourse import bacc, mybir
from concourse.bass_utils import run_bass_kernel_spmd

B = 32          # batch
S = 512         # seq len / number of diagonals
N_CORES = 8
N_SLOTS = S // N_CORES  # 64 slots per core
DCOL = B        # width of the D^T block (batch on matmul M axis)
GROUP = 4       # slots per psum group
N_GROUPS = N_SLOTS // GROUP
WF = 3072       # window free size (elems per partition)

KCFG = {
    "compute": "bf16",  # "f32" | "f32r" | "bf16"
    "out": "f32",       # "f32" | "bf16"
    "win_bufs": 12,
    "psum_bufs": 2,
}

import os as _os  # noqa: E402

# KSTACK: 0 = no stacking, 1 = full stacking (pb 32+64), 2 = B-pairs only (pb 64)
_STACK = int(_os.environ.get("KSTACK", "1"))

# ---- static layout ----------------------------------------------------
# processing order: largest slot first
_ORDER = list(range(N_SLOTS - 1, -1, -1))
_GROUPS = [_ORDER[g * GROUP : (g + 1) * GROUP] for g in range(N_GROUPS)]


# window capacity: small first windows so the first matmuls start early
def _wcap(w):
    return (1024, 2048)[w] if w < 2 else WF


# chunk placement. Full 128-row chunks get their own [128, wd] column
# block. Partial chunks are stacked vertically where the SBUF AP base
# partition ({0,32,64} only) and matmul tile grid (rows<=32 at any of
# those, rows<=64 at {0,64}) allow: with j = 16q + t the partial has
# 8(t+1) rows (t=15: none), so
#   t 5,7  (48/64 rows, pb=0) hosts partner t-1 (<=56 rows) at pb=64
#   t 3    (32 rows, pb=0) hosts t-1 (24 rows) at pb=32 and
#          t-2 (16 rows) at pb=64
#   t 0, 8..14 (8 rows / >64 rows) stay alone
# Partners are processed <=1 psum group after their primary, so the
# shared window only stays live a little longer (covered by win_bufs).
_SLOT_CHUNKS = {j: [] for j in range(N_SLOTS)}  # j -> [(win, cbase, pbase, rows, row_start)]
_cur_win = 0
_cur_col = 0


def _new_block(wd):
    global _cur_win, _cur_col
    if _cur_col + wd > _wcap(_cur_win):
        _cur_win += 1
        _cur_col = 0
    blk = (_cur_win, _cur_col)
    _cur_col += wd
    return blk


def _partial_rows(j):
    return (8 * (j + 1)) % 128


for _j in _ORDER:
    _NJ = 8 * (_j + 1)
    _wd = DCOL + _NJ
    _t = _j % 16
    _nfull = _NJ // 128
    for _c in range(_nfull):
        _w, _cb = _new_block(_wd)
        _SLOT_CHUNKS[_j].append((_w, _cb, 0, 128, 128 * _c))
    _pr = _partial_rows(_j)
    if _pr == 0:
        continue
    if _STACK == 0:
        _w, _cb = _new_block(_wd)
        _SLOT_CHUNKS[_j].append((_w, _cb, 0, _pr, 128 * _nfull))
    elif _t in (5, 7):
        _w, _cb = _new_block(_wd)
        _SLOT_CHUNKS[_j].append((_w, _cb, 0, _pr, 128 * _nfull))
        _pj = _j - 1
        _SLOT_CHUNKS[_pj].append(
            (_w, _cb, 64, _partial_rows(_pj), 128 * ((8 * (_pj + 1)) // 128))
        )
    elif _t == 3 and _STACK == 1:
        _w, _cb = _new_block(_wd)
        _SLOT_CHUNKS[_j].append((_w, _cb, 0, _pr, 128 * _nfull))
        for _pj, _ppb in ((_j - 1, 32), (_j - 2, 64)):
            _SLOT_CHUNKS[_pj].append(
                (_w, _cb, _ppb, _partial_rows(_pj), 128 * ((8 * (_pj + 1)) // 128))
            )
    elif _t == 3 and _STACK == 2:
        # pb=32 disabled: pair t3@0 with t2@64 only
        _w, _cb = _new_block(_wd)
        _SLOT_CHUNKS[_j].append((_w, _cb, 0, _pr, 128 * _nfull))
        _pj = _j - 1
        _SLOT_CHUNKS[_pj].append(
            (_w, _cb, 64, _partial_rows(_pj), 128 * ((8 * (_pj + 1)) // 128))
        )
    elif _t in (0, 8, 9, 10, 11, 12, 13, 14) or (_t == 1 and _STACK == 2):
        _w, _cb = _new_block(_wd)
        _SLOT_CHUNKS[_j].append((_w, _cb, 0, _pr, 128 * _nfull))
    # t in (1, 2, 4, 6): partial already placed by its stacking primary

N_WINS = _cur_win + 1
# exact used width per window (ship no window-tail padding)
_WIN_W = [0] * N_WINS
for _j, _chs in _SLOT_CHUNKS.items():
    _wd = DCOL + 8 * (_j + 1)
    for _w, _cb, _pb, _rows, _rs in _chs:
        _WIN_W[_w] = max(_WIN_W[_w], _cb + _wd)
_WIN_OFF = []
_boff = 0
for _w in range(N_WINS):
    _WIN_OFF.append(_boff)
    _boff += 128 * _WIN_W[_w]
BLOB_ELEMS = _boff

# psum group column layout (bank-aligned, no matmul straddles a bank)
_BANK = 512
_GROUP_COLS = []
_GROUP_W = []
for _slots in _GROUPS:
    _col = 0
    _cols = []
    for _j in _slots:
        _NJ = 8 * (_j + 1)
        if _col // _BANK != (_col + _NJ - 1) // _BANK:
            _col = ((_col + _BANK - 1) // _BANK) * _BANK
        _cols.append((_j, _col))
        _col += _NJ
    _GROUP_COLS.append(_cols)
    _GROUP_W.append(_col)

_GOUT_OFF = []
_SLOT_OUT = {}
_goff = 0
for _g in range(N_GROUPS):
    _GOUT_OFF.append(_goff)
    for _j, _col in _GROUP_COLS[_g]:
        _SLOT_OUT[_j] = (_g, _col)
    _goff += B * _GROUP_W[_g]
OUT_ELEMS = _goff

_compiled = {}


def _build_program():
    key = (KCFG["compute"], KCFG["out"], KCFG["win_bufs"], KCFG["psum_bufs"])
    if key in _compiled:
        return _compiled[key]

    from contextlib import ExitStack

    nc = bacc.Bacc("TRN2", target_bir_lowering=False, debug=False)
    f32 = mybir.dt.float32
    mm_dt = {
        "f32": f32,
        "f32r": mybir.dt.float32r,
        "bf16": mybir.dt.bfloat16,
    }[KCFG["compute"]]
    out_dt = {"f32": f32, "bf16": mybir.dt.bfloat16}[KCFG["out"]]
    blob = nc.dram_tensor("blob", [BLOB_ELEMS], mm_dt, kind="ExternalInput").ap()
    outb = nc.dram_tensor("outblob", [OUT_ELEMS], out_dt, kind="ExternalOutput").ap()

    with tile.TileContext(nc) as tc, ExitStack() as ctx:
        win_pool = ctx.enter_context(
            tc.tile_pool(name="win", bufs=KCFG["win_bufs"])
        )
        acc_pool = ctx.enter_context(tc.tile_pool(name="acc", bufs=1))
        psum_pool = ctx.enter_context(
            tc.tile_pool(name="psum", bufs=KCFG["psum_bufs"], space="PSUM")
        )

        # window tiles are loaded lazily in program order; keep handles
        win_tiles = [None] * N_WINS

        def ensure_win(w):
            if win_tiles[w] is None:
                wf = _WIN_W[w]
                t = win_pool.tile([128, wf], mm_dt)
                src = blob[_WIN_OFF[w] : _WIN_OFF[w] + 128 * wf].rearrange(
                    "(p f) -> p f", p=128, f=wf
                )
                nc.gpsimd.dma_start(t[:], src)
                win_tiles[w] = t
            return win_tiles[w]

        tot_w = OUT_ELEMS // B
        acc_t = acc_pool.tile([B, tot_w], out_dt)
        for g, slots in enumerate(_GROUPS):
            gw = _GROUP_W[g]
            gcol = _GOUT_OFF[g] // B
            psum_t = psum_pool.tile([B, gw], f32)
            for j, col in _GROUP_COLS[g]:
                NJ = 8 * (j + 1)
                wd = DCOL + NJ
                chs = _SLOT_CHUNKS[j]
                for c, (w, cb, pb, rows, _rs) in enumerate(chs):
                    t = ensure_win(w)
                    nc.tensor.matmul(
                        psum_t[:, col : col + NJ],
                        t[pb : pb + rows, cb : cb + DCOL],
                        t[pb : pb + rows, cb + DCOL : cb + wd],
                        start=(c == 0),
                        stop=(c == len(chs) - 1),
                    )
            nc.vector.tensor_copy(acc_t[:, gcol : gcol + gw], psum_t[:])
        # staged stores: earlier group ranges flush while later compute
        # still runs. All after the loads on the Pool queue, so a store
        # wait only ever blocks later (even more dependent) stores.
        dstv = outb[:].rearrange("(p w) -> p w", p=B, w=tot_w)
        cuts = [0, _GOUT_OFF[8] // B, _GOUT_OFF[13] // B, tot_w]
        for a, bnd in zip(cuts, cuts[1:]):
            nc.gpsimd.dma_start(dstv[:, a:bnd], acc_t[:, a:bnd])

    nc.compile()
    _compiled[key] = nc
    return nc


def _np_dt():
    if KCFG["compute"] == "bf16":
        import ml_dtypes

        return ml_dtypes.bfloat16
    return np.float32


def _pack_core(k, x, W, bias):
    np_dt = _np_dt()
    blob = np.zeros(BLOB_ELEMS, np_dt)
    for j in range(N_SLOTS):
        i = N_CORES * j + k
        ni = i + 1
        NJ = 8 * (j + 1)
        wd = DCOL + NJ
        M = np.zeros((NJ, wd), np.float32)
        r = np.arange(ni)
        M[:ni, :DCOL] = x[:, r, i - r].T               # D^T[r, b]
        M[:ni, DCOL : DCOL + ni] = W[i, :ni, :ni].T    # V[r, q]
        for w, cb, pb, rows, rs in _SLOT_CHUNKS[j]:
            rl = M[rs : rs + rows]                     # [rows, wd]
            wf = _WIN_W[w]
            img = blob[_WIN_OFF[w] : _WIN_OFF[w] + 128 * wf].reshape(128, wf)
            img[pb : pb + rows, cb : cb + wd] = rl.astype(np_dt)
    return blob


def kernel(x, W, b):
    x = np.asarray(x, np.float32)
    W = np.asarray(W, np.float32)
    b = np.asarray(b, np.float32)

    nc = _build_program()
    in_maps = [{"blob": _pack_core(k, x, W, b)} for k in range(N_CORES)]
    res = run_bass_kernel_spmd(nc, in_maps, list(range(N_CORES)))

    y = x.copy()
    tot_w = OUT_ELEMS // B
    for k in range(N_CORES):
        ob = np.asarray(res.results[k]["outblob"], np.float32).reshape(B, tot_w)
        for j in range(N_SLOTS):
            i = N_CORES * j + k
            ni = i + 1
            g, col = _SLOT_OUT[j]
            gcol = _GOUT_OFF[g] // B + col
            q = np.arange(ni)
            y[:, q, i - q] = ob[:, gcol : gcol + ni] + b[i, :ni][None]
    return y
